# revision 10
# baseline (speedup 1.0000x reference)
"""ConformerBlock Trainium2 kernel (Bass/Tile), 8-core SPMD — v2.

Sharding: core c handles batch b=c//2, sequence half c%2.  Half-1 cores
receive the sequence REVERSED (and reversed conv taps) so that every core's
program is identical: own tokens are positions [0,1024), the query/conv
window is positions [0,1152), conv zero-padding is on the left edge.
Attention keys are order-invariant; the depthwise conv commutes with
reversal when taps are reversed; everything else is per-token.

ffn1 + qkv run redundantly over the full 2048-token batch so attention K/V
need no cross-core communication.

v2 numerics: act_quant's round-trip round(x*s)/s is per-token quantization
noise on a relative grid; it is approximated by a plain bf16 cast (same
relative step 2^-9, scale-invariant), which eliminates all per-token
absmax/scale work.  Ternary weights are exact in bf16; matmuls accumulate
in fp32 PSUM.  Validated end-to-end in numpy: rel err 2.5e-3 vs reference.

v2 layout: BitLinear branches run FEATURE-major internally (out = W @ x
via lhsT=W-chunk stationary, rhs=x^T streaming), so the FFN hidden is
never transposed; snake runs with per-partition (=per-feature) activation
scales.  Only branch inputs/outputs are transposed (bf16, 1 cyc/row).
The depthwise conv consumes the feature-major GLU output directly.
"""

from contextlib import ExitStack

import ml_dtypes
import numpy as np

import concourse.bass as bass
import concourse.mybir as mybir
import concourse.tile as tile
from concourse.bass import ts
from concourse.masks import make_identity
import json as _json


def _fix_bir(nc):
    """This container's walrus allows at most ONE sem wait per instruction.
    Hoist surplus waits: for engine instructions onto injected same-engine
    NoOps; for DMACopies onto prepended 1-element dummy copies on the same
    queue (ring order gates the real transfer, identical semantics)."""
    orig = nc.to_json_bytes

    def patched():
        import copy as _copy
        data = _json.loads(orig())
        used = set()
        for fn in data["functions"]:
            for bb in fn["blocks"]:
                for ins in bb["instructions"]:
                    si2 = ins.get("sync_info") or {}
                    for w in (si2.get("on_wait") or []):
                        used.add(w.get("id", 0))
                    for u in (si2.get("on_update") or []):
                        used.add(u.get("id", 0))
        scratch_sem = max(used) + 1 if used else 60
        k = 0
        for fn in data["functions"]:
            for bb in fn["blocks"]:
                out = []
                for ins in bb["instructions"]:
                    si = ins.get("sync_info")
                    ow = (si or {}).get("on_wait") or []
                    if len(ow) > 1:
                        if ins.get("opcode") == "DMACopy":
                            for w in ow[:-1]:
                                k += 1
                                d = _copy.deepcopy(ins)
                                d["name"] = f"W-{k}"
                                d["sync_info"] = {
                                    "on_wait": [w],
                                    "on_update": [{
                                        "ant_name": "WFIX_scratch",
                                        "id": scratch_sem,
                                        "sync_type": "semaphore",
                                        "update_mode": "sem-inc",
                                        "update_value": 1}]}
                                for ap in list(d.get("ins", [])) + list(d.get("outs", [])):
                                    if isinstance(ap, dict) and "ap" in ap:
                                        ap["ap"] = [[s, 1] for s, _ in ap["ap"]]
                                out.append(d)
                            si["on_wait"] = [ow[-1]]
                        else:
                            for w in ow[:-1]:
                                k += 1
                                nop = {"name": f"W-{k}",
                                       "engine": ins["engine"],
                                       "opcode": "NoOp", "ins": [],
                                       "outs": [],
                                       "sync_info": {"on_wait": [w]}}
                                if "debug" in ins:
                                    nop["debug"] = ins["debug"]
                                out.append(nop)
                            si["on_wait"] = [ow[-1]]
                    out.append(ins)
                bb["instructions"] = out
        return _json.dumps(data).encode()

    nc.to_json_bytes = patched
    return nc


ml_bf16 = ml_dtypes.bfloat16

P = 128
T = 2048          # tokens per batch
D = 512           # model dim
FF = 2048         # ffn hidden
H = 8             # heads
HD = 64           # head dim
KW = 31           # conv kernel
QN = 1152         # per-core query window (9 tiles), covers own 1024 + halo 15
OWN = 1024
NT = T // P       # 16
NQ = QN // P      # 9
NO = OWN // P     # 8
ND = D // P       # 4
NF = FF // P      # 16
CH = 512          # token chunk (matmul N)
CIN_W = 15 + 1039 + 2  # conv input row: cols j <-> token j-15; pad to 1056

F32 = mybir.dt.float32
BF16 = mybir.dt.bfloat16
AX = mybir.AxisListType.X
OP = mybir.AluOpType
AF = mybir.ActivationFunctionType


# ---------------------------------------------------------------- host prep

def np_w_quant(w):
    """Host replica of reference w_quant: (ternary int8, descale 1/s)."""
    s = np.float32(1.0) / np.clip(
        np.abs(w).mean(dtype=np.float32), np.float32(1e-5), None
    ).astype(np.float32)
    q = np.clip(np.round(w * s), -1, 1).astype(np.int8)
    return q, np.float32(1.0) / s


class Spec:
    """Host-side preprocessing of all parameters (shared across cores)."""

    def __init__(self, inp):
        f32 = np.float32

        def prep(w, kt):
            # ternary, transposed to lhsT layout [K_in, M_out] -> [kt,128,M]
            q, dsc = np_w_quant(np.asarray(w, f32))
            wt = np.ascontiguousarray(q.T).astype(ml_bf16)
            return wt.reshape(kt, P, wt.shape[1]), f32(dsc)

        self.w1a, self.d_w1a = prep(inp["ff1_w1"], ND)    # [4,128,2048]
        self.w2a, self.d_w2a = prep(inp["ff1_w2"], NF)    # [16,128,512]
        self.w1b, self.d_w1b = prep(inp["ff2_w1"], ND)
        self.w2b, self.d_w2b = prep(inp["ff2_w2"], NF)
        self.pw1, self.d_pw1 = prep(inp["pw1_w"], ND)     # [4,128,1024]
        self.pw2, self.d_pw2 = prep(inp["pw2_w"], ND)     # [4,128,512]

        ipw = np.asarray(inp["in_proj_w"], f32)           # [1536, 512]
        self.wqk = np.ascontiguousarray(ipw[: 2 * D].T).astype(ml_bf16).reshape(ND, P, 2 * D)
        self.wv = np.ascontiguousarray(ipw[2 * D:].T).astype(ml_bf16).reshape(ND, P, D)
        self.opw = np.ascontiguousarray(np.asarray(inp["out_proj_w"], f32).T).astype(ml_bf16).reshape(ND, P, D)

        self.ipb = np.asarray(inp["in_proj_b"], f32)
        self.opb = np.asarray(inp["out_proj_b"], f32)
        self.has_ipb = bool(np.any(self.ipb != 0))
        self.has_opb = bool(np.any(self.opb != 0))

        self.nw = {}
        self.has_nw = {}
        for k in ("ff1_norm_w", "attn_norm_w", "conv_norm_w", "ff2_norm_w",
                  "final_norm_w"):
            w = np.asarray(inp[k], f32)
            self.nw[k] = w
            self.has_nw[k] = bool(np.any(w != 1.0))

        def snake(la, lb, dsc):
            # a_eff = a*dsc (sin scale, folded descale); sqivb = sqrt(ivb)
            a = np.exp(np.asarray(la, f32)).astype(f32)
            ivb = (f32(1.0) / (np.exp(np.asarray(lb, f32)) + f32(1e-9))).astype(f32)
            has_a = bool(np.any(a != 1.0))
            has_ivb = bool(np.any(np.abs(ivb - 1.0) > 1e-7))
            return (a * dsc).astype(f32), np.sqrt(ivb).astype(f32), has_a, has_ivb

        self.a1, self.sqivb1, self.has_a1, self.has_ivb1 = snake(
            inp["ff1_a"], inp["ff1_b"], self.d_w1a)
        self.a2, self.sqivb2, self.has_a2, self.has_ivb2 = snake(
            inp["ff2_a"], inp["ff2_b"], self.d_w1b)
        self.a3, self.sqivb3, self.has_a3, self.has_ivb3 = snake(
            inp["snake_a"], inp["snake_b"], np.float32(1.0))

        # depthwise conv folded with batchnorm:
        # y = conv(glu)*A + B,  A = g*rsqrt(v+1e-5), B = (dwb-m)*A + b
        A = (np.asarray(inp["bn_g"], f32)
             / np.sqrt(np.asarray(inp["bn_v"], f32) + f32(1e-5))).astype(f32)
        Bb = ((np.asarray(inp["dw_b"], f32) - np.asarray(inp["bn_m"], f32)) * A
              + np.asarray(inp["bn_b"], f32)).astype(f32)
        dw = np.asarray(inp["dw_w"], f32)[:, 0, :]        # [512, 31]
        self.wA = (dw * A[:, None]).astype(f32)           # [512, 31]
        self.convB = Bb.reshape(ND, P)                    # [4, 128]


# ------------------------------------------------------------- device build

class Ctx:
    def __init__(self, nc, tc, st):
        self.nc, self.tc, self.st = nc, tc, st


def _batched_stats(c, pool, src, ntiles):
    """Per-tile sum(x^2) over token tiles of src [128, ntiles, 512]."""
    nc = c.nc
    ssum = pool.tile([P, ntiles], F32, tag="st_ssum", name="st_ssum")
    sq = pool.tile([P, D], F32, tag="st_sq", name="st_sq", bufs=2)
    for i in range(ntiles):
        nc.vector.tensor_mul(sq, src[:, i, :], src[:, i, :])
        nc.vector.tensor_reduce(ssum[:, i:i + 1], sq, AX, OP.add)
    return ssum


def _rsqrt_newton(c, pool, ssum, n_el, eps=1e-6):
    """rs = rsqrt(ssum/n_el + eps) with one Newton step."""
    nc = c.nc
    n = ssum.shape[-1]
    m = pool.tile([P, n], F32, tag="st_m", name="st_m")
    nc.vector.tensor_scalar(out=m, in0=ssum, scalar1=1.0 / n_el, scalar2=eps,
                            op0=OP.mult, op1=OP.add)
    rc = pool.tile([P, n], F32, tag="st_rc", name="st_rc")
    nc.vector.reciprocal(rc, m)
    rs = pool.tile([P, n], F32, tag="st_rs", name="st_rs")
    nc.scalar.activation(out=rs, in_=rc, func=AF.Sqrt)
    t1 = pool.tile([P, n], F32, tag="st_t1", name="st_t1")
    nc.vector.tensor_mul(t1, rs, rs)
    nc.vector.tensor_mul(t1, t1, m)
    nc.vector.tensor_scalar(out=t1, in0=t1, scalar1=-0.5, scalar2=1.5,
                            op0=OP.mult, op1=OP.add)
    nc.vector.tensor_mul(rs, rs, t1)
    return rs


def _norm_cast_T(c, pools, src, ntiles, nw_b, ident, out_tag="xqT"):
    """rmsnorm (approx act_quant = bf16 cast) + transpose.
    src [128, ntiles, 512] f32 -> xqT [128, 4, ntiles*128] bf16."""
    nc = c.nc
    pool, scratch, pst = pools
    ssum = _batched_stats(c, pool, src, ntiles)
    rs = _rsqrt_newton(c, pool, ssum, D)
    xqT = pool.tile([P, ND, ntiles * P], BF16, tag=out_tag, name=out_tag)
    for i in range(ntiles):
        xq = scratch.tile([P, D], BF16, tag="nc_xq", name="nc_xq")
        if nw_b is None:
            nc.vector.tensor_scalar_mul(xq, src[:, i, :], rs[:, i:i + 1])
        else:
            nc.vector.scalar_tensor_tensor(
                out=xq, in0=src[:, i, :], scalar=rs[:, i:i + 1], in1=nw_b,
                op0=OP.mult, op1=OP.mult)
        pt = pst.tile([P, ND, P], BF16, tag="pt", name="pt")
        for dt in range(ND):
            nc.tensor.transpose(pt[:, dt, :], xq[:, ts(dt, P)], ident)
        nc.vector.tensor_copy(xqT[:, :, ts(i, P)], pt)
    return xqT


def _ffn(c, pools, src, ntiles, w1, w2, d1, d2, a_fm, sqivb_fm, has_a,
         has_ivb, nw_b, ident, resid_scale, dst):
    """dst = src + resid_scale * ffn(src).  Feature-major hidden.
    src/dst: [128, ntiles, 512] f32 token-major."""
    nc = c.nc
    pool, scratch, pst, psH, psO = pools
    nch = ntiles * P // CH
    xqT = _norm_cast_T(c, (pool, scratch, pst), src, ntiles, nw_b, ident)
    d1f = float(d1)
    dof = float(d2) * float(resid_scale)
    for j in range(nch):
        hq = scratch.tile([P, NF, CH], BF16, tag="ffn_hq", name="ffn_hq")
        for g in range(NF // 2):
            ph = psH.tile([P, 2, CH], F32, tag="ph", name="ph")
            for f2 in range(2):
                fc = 2 * g + f2
                for kt in range(ND):
                    nc.tensor.matmul(ph[:, f2, :], w1[:, kt, ts(fc, P)],
                                     xqT[:, kt, ts(j, CH)],
                                     start=(kt == 0), stop=(kt == ND - 1))
            phv = ph.rearrange("p a b -> p (a b)")
            sn = scratch.tile([P, 2 * CH], F32, tag="ffn_sn", name="ffn_sn")
            if not has_a:
                nc.scalar.activation(out=sn, in_=phv, func=AF.Sin, scale=d1f)
            else:
                for f2 in range(2):
                    fc = 2 * g + f2
                    nc.scalar.activation(out=sn[:, ts(f2, CH)],
                                         in_=ph[:, f2, :], func=AF.Sin,
                                         scale=a_fm[:, fc:fc + 1])
            sq = scratch.tile([P, 2 * CH], F32, tag="ffn_sq", name="ffn_sq")
            if not has_ivb:
                nc.scalar.activation(out=sq, in_=sn, func=AF.Square)
            else:
                for f2 in range(2):
                    fc = 2 * g + f2
                    nc.scalar.activation(out=sq[:, ts(f2, CH)],
                                         in_=sn[:, ts(f2, CH)],
                                         func=AF.Square,
                                         scale=sqivb_fm[:, fc:fc + 1])
            hqv = hq[:, 2 * g:2 * g + 2, :].rearrange("p a b -> p (a b)")
            nc.vector.scalar_tensor_tensor(out=hqv, in0=phv, scalar=d1f,
                                           in1=sq, op0=OP.mult, op1=OP.add)
        for dc in range(ND):
            p2 = psO.tile([P, CH], F32, tag="p2", name="p2")
            for kt in range(NF):
                nc.tensor.matmul(p2, w2[:, kt, ts(dc, P)], hq[:, kt, :],
                                 start=(kt == 0), stop=(kt == NF - 1))
            ob = scratch.tile([P, CH], BF16, tag="ffn_ob", name="ffn_ob")
            nc.vector.tensor_copy(ob, p2)
            pt = pst.tile([P, ND, P], BF16, tag="pt", name="pt")
            for tc in range(4):
                nc.tensor.transpose(pt[:, tc, :], ob[:, ts(tc, P)], ident)
            for tc in range(4):
                i = 4 * j + tc
                nc.vector.scalar_tensor_tensor(
                    out=dst[:, i, ts(dc, P)], in0=pt[:, tc, :], scalar=dof,
                    in1=src[:, i, ts(dc, P)], op0=OP.mult, op1=OP.add)


def build(spec: Spec, debug=False):
    nc = bass.Bass()
    st = spec
    if st.has_ipb:
        raise NotImplementedError("nonzero in_proj_b not supported")

    # ---- dram params
    x_d = nc.declare_dram_parameter("x", [T, D], F32, isOutput=False)
    w_names = {}
    for nm, arr in [("w1a", st.w1a), ("w2a", st.w2a), ("w1b", st.w1b),
                    ("w2b", st.w2b), ("pw1", st.pw1), ("pw2", st.pw2),
                    ("wqk", st.wqk), ("wv", st.wv), ("opw", st.opw)]:
        w_names[nm] = nc.declare_dram_parameter(nm, list(arr.shape), BF16,
                                                isOutput=False)
    wA_d = nc.declare_dram_parameter("wA", [ND, P, KW], F32, isOutput=False)
    convB_d = nc.declare_dram_parameter("convB", [ND, P], F32, isOutput=False)
    vec_d = {}
    for nm, need, n in [("a1", st.has_a1, FF), ("sqivb1", st.has_ivb1, FF),
                        ("a2", st.has_a2, FF), ("sqivb2", st.has_ivb2, FF),
                        ("a3", st.has_a3, D), ("sqivb3", st.has_ivb3, D),
                        ("opb", st.has_opb, D)]:
        if need:
            vec_d[nm] = nc.declare_dram_parameter(nm, [n], F32, isOutput=False)
    nwflags = ["ff1_norm_w", "attn_norm_w", "conv_norm_w", "ff2_norm_w",
               "final_norm_w"]
    for k in nwflags:
        if st.has_nw[k]:
            vec_d[k] = nc.declare_dram_parameter(k, [D], F32, isOutput=False)

    out_d = nc.declare_dram_parameter("out", [OWN, D], F32, isOutput=True)
    if debug:
        dbg1 = nc.declare_dram_parameter("dbg_x1", [T, D], F32, isOutput=True)
        dbg2 = nc.declare_dram_parameter("dbg_x2", [QN, D], F32, isOutput=True)
        dbg3 = nc.declare_dram_parameter("dbg_x3", [OWN, D], F32, isOutput=True)

    def bcast_load(pool, dram_ap, n, tag):
        t = pool.tile([P, n], F32, tag=tag, name=tag)
        src = bass.AP(tensor=dram_ap.tensor, offset=dram_ap.offset,
                      ap=[[0, P]] + dram_ap.ap)
        nc.sync.dma_start(out=t, in_=src)
        return t

    def fm_load(pool, dram_ap, ncol, tag):
        # [ncol*128] vector -> [128, ncol] feature-major tile
        t = pool.tile([P, ncol], F32, tag=tag, name=tag)
        nc.sync.dma_start(out=t, in_=dram_ap.rearrange("(a p) -> p a", p=P))
        return t

    def load_w(pool, nm):
        arr = getattr(st, nm)
        t = pool.tile([P, arr.shape[0], arr.shape[2]], BF16,
                      tag=f"w_{nm}", name=f"w_{nm}")
        for kt in range(arr.shape[0]):
            nc.sync.dma_start(out=t[:, kt, :], in_=w_names[nm][kt])
        return t

    with tile.TileContext(nc) as tc:
        c = Ctx(nc, tc, st)
        with ExitStack() as es:
            glob = es.enter_context(tc.tile_pool(name="glob", bufs=1))
            pst = es.enter_context(tc.tile_pool(name="pst", bufs=2,
                                                space="PSUM"))

            ident = glob.tile([P, P], BF16)
            make_identity(nc, ident)

            a1_fm = fm_load(glob, vec_d["a1"][:], NF, "a1fm") if st.has_a1 else None
            sqivb1_fm = fm_load(glob, vec_d["sqivb1"][:], NF, "sqivb1fm") if st.has_ivb1 else None
            a2_fm = fm_load(glob, vec_d["a2"][:], NF, "a2fm") if st.has_a2 else None
            sqivb2_fm = fm_load(glob, vec_d["sqivb2"][:], NF, "sqivb2fm") if st.has_ivb2 else None
            a3_fm = fm_load(glob, vec_d["a3"][:], ND, "a3fm") if st.has_a3 else None
            sqivb3_fm = fm_load(glob, vec_d["sqivb3"][:], ND, "sqivb3fm") if st.has_ivb3 else None
            opb_b = bcast_load(glob, vec_d["opb"][:], D, "opbb") if st.has_opb else None
            nw_b = {k: (bcast_load(glob, vec_d[k][:], D, f"nw_{k}")
                        if st.has_nw[k] else None) for k in nwflags}

            # persistent residual-stream tiles (updated in place)
            X = glob.tile([P, NT, D], F32)     # x, then x1 after phase A
            x2 = glob.tile([P, NQ, D], F32)    # x2, then x3/x4 in [:, :8, :]

            xr = x_d[:].rearrange("(t p) d -> p t d", p=P)
            for i in range(NT):
                nc.sync.dma_start(out=X[:, i, :], in_=xr[:, i, :])

            # ---------------- phase A: ffn1 over full batch; X <- x1
            with tc.tile_pool(name="poolA", bufs=1) as pool, \
                 tc.tile_pool(name="scrA", bufs=2) as scratch, \
                 tc.tile_pool(name="psHA", bufs=2, space="PSUM") as psH, \
                 tc.tile_pool(name="psOA", bufs=2, space="PSUM") as psO:
                w1 = load_w(pool, "w1a")
                w2 = load_w(pool, "w2a")
                _ffn(c, (pool, scratch, pst, psH, psO), X, NT,
                     w1, w2, st.d_w1a, st.d_w2a, a1_fm, sqivb1_fm,
                     st.has_a1, st.has_ivb1, nw_b["ff1_norm_w"], ident,
                     0.5, X)
            if debug:
                d1r = dbg1[:].rearrange("(t p) d -> p t d", p=P)
                for i in range(NT):
                    nc.sync.dma_start(out=d1r[:, i, :], in_=X[:, i, :])

            # ---------------- phase B: attention -> x2 (window [0,1152))
            with tc.tile_pool(name="poolB", bufs=1) as pool, \
                 tc.tile_pool(name="scrB", bufs=2) as scratch:
                wqk = load_w(pool, "wqk")
                wv = load_w(pool, "wv")
                opw = load_w(pool, "opw")

                # B1: rmsnorm(x1) -> bf16, transposed
                xn2T = _norm_cast_T(c, (pool, scratch, pst), X, NT,
                                    nw_b["attn_norm_w"], ident,
                                    out_tag="xn2T")

                # B2: q,k feature-major; B3: v token-major + ones cols
                qkT = pool.tile([P, H, T], BF16)
                V = pool.tile([P, NT, H * (HD + 1)], BF16)
                nc.vector.memset(V, 1.0)
                with tc.tile_pool(name="psB", bufs=1, space="PSUM") as psB, \
                     tc.tile_pool(name="psB2", bufs=2, space="PSUM") as psB2:
                    for mt in range(H):
                        pq = psB.tile([P, ND, D], F32, tag="ps_qk",
                                      name="ps_qk")
                        for nc_ in range(ND):
                            for kt in range(ND):
                                nc.tensor.matmul(
                                    pq[:, nc_, :], wqk[:, kt, ts(mt, P)],
                                    xn2T[:, kt, ts(nc_, D)],
                                    start=(kt == 0), stop=(kt == ND - 1))
                        pqv = pq.rearrange("p a b -> p (a b)")
                        nc.vector.tensor_copy(qkT[:, mt, :], pqv)
                    for i in range(NT):
                        pv = psB2.tile([P, D], F32, tag="ps_v", name="ps_v")
                        for kt in range(ND):
                            nc.tensor.matmul(pv, xn2T[:, kt, ts(i, P)],
                                             wv[:, kt, :],
                                             start=(kt == 0),
                                             stop=(kt == ND - 1))
                        vv = V[:, i, :].rearrange("p (h z) -> p h z", z=HD + 1)
                        nc.vector.tensor_copy(
                            vv[:, :, 0:HD],
                            pv.rearrange("p (h z) -> p h z", z=HD))

                # B4: attention per head over query window [0, QN)
                On = pool.tile([P, ND, QN], BF16)
                QCH = [(0, 512), (512, 512), (1024, 128)]
                with tc.tile_pool(name="psS", bufs=1, space="PSUM") as psS, \
                     tc.tile_pool(name="psO", bufs=2, space="PSUM") as psO, \
                     tc.tile_pool(name="drB", bufs=2, space="DRAM") as drB:
                    for h in range(H):
                        kf_t, kf_o = ND + h // 2, (h % 2) * HD
                        qf_t, qf_o = h // 2, (h % 2) * HD
                        for (q0, qw) in QCH:
                            PT = scratch.tile([P, NT, 512], BF16, tag="PT",
                                              name="PT")
                            for ktg in range(4):
                                ps = psS.tile([P, 4, 512], F32, tag="ps_s",
                                              name="ps_s")
                                for k4 in range(4):
                                    kt = ktg * 4 + k4
                                    nc.tensor.matmul(
                                        ps[:, k4, :qw],
                                        qkT[kf_o:kf_o + HD, kf_t, ts(kt, P)],
                                        qkT[qf_o:qf_o + HD, qf_t, q0:q0 + qw],
                                        start=True, stop=True)
                                nc.scalar.activation(
                                    out=PT[:, ktg * 4:(ktg + 1) * 4, :qw],
                                    in_=ps[:, :, :qw], func=AF.Exp,
                                    scale=0.125)
                            po = psO.tile([HD + 1, 512], F32, tag="ps_o",
                                          name="ps_o")
                            for kt in range(NT):
                                nc.tensor.matmul(
                                    po[:, :qw],
                                    V[:, kt, h * (HD + 1):(h + 1) * (HD + 1)],
                                    PT[:, kt, :qw],
                                    start=(kt == 0), stop=(kt == NT - 1))
                            # normalize rows 0:64 by row 64 (denominator):
                            # reciprocal on scalar engine, partition-broadcast
                            # via SBUF->SBUF DMA, multiply on DVE.
                            dn = scratch.tile([1, 512], F32, tag="dn",
                                              name="dn")
                            nc.vector.reciprocal(dn[:, :qw],
                                                 po[HD:HD + 1, :qw])
                            bounce = drB.tile([1, 512], F32, tag="bnc",
                                              name="bnc")
                            nc.sync.dma_start(out=bounce[:, :qw],
                                              in_=dn[:, :qw])
                            rd = scratch.tile([HD, 512], F32, tag="rd",
                                              name="rd")
                            bap = bass.AP(
                                tensor=bounce.tensor, offset=bounce.offset,
                                ap=[[0, HD]] + bounce[:, :qw].ap[1:])
                            nc.sync.dma_start(out=rd[:, :qw], in_=bap)
                            nc.vector.tensor_mul(
                                On[qf_o:qf_o + HD, qf_t, q0:q0 + qw],
                                po[0:HD, :qw], rd[:, :qw])

                # B6: out-proj + residual -> x2
                with tc.tile_pool(name="psB6", bufs=2, space="PSUM") as psB6:
                    for i in range(NQ):
                        pp = psB6.tile([P, D], F32, tag="ps_op", name="ps_op")
                        for kt in range(ND):
                            nc.tensor.matmul(pp, On[:, kt, ts(i, P)],
                                             opw[:, kt, :],
                                             start=(kt == 0),
                                             stop=(kt == ND - 1))
                        if opb_b is not None:
                            tmp = scratch.tile([P, D], F32, tag="b6_t",
                                               name="b6_t")
                            nc.vector.tensor_add(tmp, pp, opb_b)
                            nc.vector.tensor_add(x2[:, i, :], tmp, X[:, i, :])
                        else:
                            nc.vector.scalar_tensor_tensor(
                                out=x2[:, i, :], in0=pp, scalar=1.0,
                                in1=X[:, i, :], op0=OP.mult, op1=OP.add)
            if debug:
                d2r = dbg2[:].rearrange("(t p) d -> p t d", p=P)
                for i in range(NQ):
                    nc.sync.dma_start(out=d2r[:, i, :], in_=x2[:, i, :])

            # ---------------- phase C: conv branch; x2[:, :8] <- x3
            with tc.tile_pool(name="poolC", bufs=1) as pool, \
                 tc.tile_pool(name="scrC", bufs=2) as scratch:
                pw1 = load_w(pool, "pw1")
                pw2 = load_w(pool, "pw2")
                wA_sb = pool.tile([P, ND, KW], F32)
                for kt in range(ND):
                    nc.sync.dma_start(out=wA_sb[:, kt, :], in_=wA_d[kt])
                convB_sb = pool.tile([P, ND], F32)
                nc.sync.dma_start(out=convB_sb,
                                  in_=convB_d[:].rearrange("a p -> p a"))

                xq3T = _norm_cast_T(c, (pool, scratch, pst), x2, NQ,
                                    nw_b["conv_norm_w"], ident,
                                    out_tag="xq3T")
                cin = pool.tile([P, ND, CIN_W], F32)
                nc.gpsimd.memset(cin, 0.0)

                d1f = float(st.d_pw1)
                QCH = [(0, 512), (512, 512), (1024, 128)]
                with tc.tile_pool(name="psC1", bufs=2, space="PSUM") as psC1, \
                     tc.tile_pool(name="psC2", bufs=2, space="PSUM") as psC2:
                    # pw1 + GLU, feature-major straight into cin
                    for (q0, qw) in QCH:
                        w = min(qw, 1054 - (15 + q0))
                        for ct in range(ND):
                            pp = psC1.tile([P, 2, CH], F32, tag="ps_pw1",
                                           name="ps_pw1")
                            for s, fc in ((0, ct), (1, 4 + ct)):
                                for kt in range(ND):
                                    nc.tensor.matmul(
                                        pp[:, s, :qw], pw1[:, kt, ts(fc, P)],
                                        xq3T[:, kt, q0:q0 + qw],
                                        start=(kt == 0), stop=(kt == ND - 1))
                            sig = scratch.tile([P, CH], F32, tag="c_sig",
                                               name="c_sig")
                            nc.scalar.activation(out=sig[:, :qw],
                                                 in_=pp[:, 1, :qw],
                                                 func=AF.Sigmoid, scale=d1f)
                            nc.vector.scalar_tensor_tensor(
                                out=cin[:, ct, 15 + q0:15 + q0 + w],
                                in0=pp[:, 0, :w], scalar=d1f,
                                in1=sig[:, :w], op0=OP.mult, op1=OP.mult)
                    # depthwise conv + folded bn (DVE)
                    acc = pool.tile([P, ND, OWN], F32)
                    for ct in range(ND):
                        nc.vector.tensor_scalar(
                            out=acc[:, ct, :], in0=cin[:, ct, 0:OWN],
                            scalar1=wA_sb[:, ct, 0:1],
                            scalar2=convB_sb[:, ct:ct + 1],
                            op0=OP.mult, op1=OP.add)
                        for k in range(1, KW):
                            nc.vector.scalar_tensor_tensor(
                                out=acc[:, ct, :], in0=cin[:, ct, k:k + OWN],
                                scalar=wA_sb[:, ct, k:k + 1],
                                in1=acc[:, ct, :], op0=OP.mult, op1=OP.add)
                    # snake (feature-major) -> quantized (bf16) pw2 input
                    zq = pool.tile([P, ND, OWN], BF16)
                    for ct in range(ND):
                        sn = scratch.tile([P, OWN], F32, tag="c_sn",
                                          name="c_sn")
                        sq = scratch.tile([P, OWN], F32, tag="c_sq",
                                          name="c_sq")
                        if st.has_a3:
                            nc.scalar.activation(out=sn, in_=acc[:, ct, :],
                                                 func=AF.Sin,
                                                 scale=a3_fm[:, ct:ct + 1])
                        else:
                            nc.scalar.activation(out=sn, in_=acc[:, ct, :],
                                                 func=AF.Sin)
                        if st.has_ivb3:
                            nc.scalar.activation(out=sq, in_=sn,
                                                 func=AF.Square,
                                                 scale=sqivb3_fm[:, ct:ct + 1])
                        else:
                            nc.scalar.activation(out=sq, in_=sn,
                                                 func=AF.Square)
                        nc.vector.tensor_add(zq[:, ct, :], acc[:, ct, :], sq)
                    # pw2 (feature-major) + transpose + residual
                    dof = float(st.d_pw2)
                    for j2 in range(2):
                        for dc in range(ND):
                            p2 = psC2.tile([P, CH], F32, tag="ps_pw2",
                                           name="ps_pw2")
                            for kt in range(ND):
                                nc.tensor.matmul(
                                    p2, pw2[:, kt, ts(dc, P)],
                                    zq[:, kt, ts(j2, CH)],
                                    start=(kt == 0), stop=(kt == ND - 1))
                            ob = scratch.tile([P, CH], BF16, tag="c_ob",
                                              name="c_ob")
                            nc.vector.tensor_copy(ob, p2)
                            pt = pst.tile([P, ND, P], BF16, tag="pt",
                                          name="pt")
                            for tc_ in range(4):
                                nc.tensor.transpose(pt[:, tc_, :],
                                                    ob[:, ts(tc_, P)], ident)
                            for tc_ in range(4):
                                i8 = 4 * j2 + tc_
                                nc.vector.scalar_tensor_tensor(
                                    out=x2[:, i8, ts(dc, P)],
                                    in0=pt[:, tc_, :], scalar=dof,
                                    in1=x2[:, i8, ts(dc, P)],
                                    op0=OP.mult, op1=OP.add)
            if debug:
                d3r = dbg3[:].rearrange("(t p) d -> p t d", p=P)
                for i in range(NO):
                    nc.sync.dma_start(out=d3r[:, i, :], in_=x2[:, i, :])

            # ---------------- phase D: ffn2; x2[:, :8] <- x4
            with tc.tile_pool(name="poolD", bufs=1) as pool, \
                 tc.tile_pool(name="scrD", bufs=2) as scratch, \
                 tc.tile_pool(name="psHD", bufs=2, space="PSUM") as psH, \
                 tc.tile_pool(name="psOD", bufs=2, space="PSUM") as psO:
                w1 = load_w(pool, "w1b")
                w2 = load_w(pool, "w2b")
                _ffn(c, (pool, scratch, pst, psH, psO), x2, NO,
                     w1, w2, st.d_w1b, st.d_w2b, a2_fm, sqivb2_fm,
                     st.has_a2, st.has_ivb2, nw_b["ff2_norm_w"], ident,
                     0.5, x2)

            # ---------------- phase E: final rmsnorm -> out
            with tc.tile_pool(name="poolE", bufs=1) as pool, \
                 tc.tile_pool(name="scrE", bufs=2) as scratch:
                ssum5 = _batched_stats(c, pool, x2, NO)
                rs5 = _rsqrt_newton(c, pool, ssum5, D)
                outr = out_d[:].rearrange("(t p) d -> p t d", p=P)
                for i in range(NO):
                    o = scratch.tile([P, D], F32, tag="e_o", name="e_o")
                    if nw_b["final_norm_w"] is None:
                        nc.vector.tensor_scalar_mul(o, x2[:, i, :],
                                                    rs5[:, i:i + 1])
                    else:
                        nc.vector.scalar_tensor_tensor(
                            out=o, in0=x2[:, i, :], scalar=rs5[:, i:i + 1],
                            in1=nw_b["final_norm_w"], op0=OP.mult,
                            op1=OP.mult)
                    nc.sync.dma_start(out=outr[:, i, :], in_=o)

    return _fix_bir(nc)


# ------------------------------------------------------------------ runner

def make_in_maps(spec: Spec, x_full):
    """x_full: [4, 2048, 512] f32.  Returns per-core input maps."""
    maps = []
    shared = {"w1a": spec.w1a, "w2a": spec.w2a, "w1b": spec.w1b,
              "w2b": spec.w2b, "pw1": spec.pw1, "pw2": spec.pw2,
              "wqk": spec.wqk, "wv": spec.wv, "opw": spec.opw}
    wA_f = spec.wA.reshape(ND, P, KW)
    wA_r = np.ascontiguousarray(spec.wA[:, ::-1]).reshape(ND, P, KW)
    opt = {}
    for nm, need, arr in [("a1", spec.has_a1, spec.a1),
                          ("sqivb1", spec.has_ivb1, spec.sqivb1),
                          ("a2", spec.has_a2, spec.a2),
                          ("sqivb2", spec.has_ivb2, spec.sqivb2),
                          ("a3", spec.has_a3, spec.a3),
                          ("sqivb3", spec.has_ivb3, spec.sqivb3),
                          ("opb", spec.has_opb, spec.opb)]:
        if need:
            opt[nm] = arr
    for k, need in spec.has_nw.items():
        if need:
            opt[k] = spec.nw[k]
    for cid in range(8):
        b, flip = cid // 2, cid % 2
        xb = x_full[b] if not flip else np.ascontiguousarray(x_full[b][::-1])
        m = {"x": np.asarray(xb, np.float32),
             "wA": wA_r if flip else wA_f,
             "convB": spec.convB, **shared, **opt}
        maps.append(m)
    return maps


def assemble_out(results):
    """results: list of 8 dicts with 'out' [1024, 512]."""
    y = np.zeros((4, T, D), np.float32)
    for cid in range(8):
        b, flip = cid // 2, cid % 2
        o = results[cid]["out"]
        if flip:
            y[b, OWN:] = o[::-1]
        else:
            y[b, :OWN] = o
    return y


# ------------------------------------------------------------------ entry

def kernel(**inputs):
    """Full-input entry point: shards across 8 NeuronCores internally."""
    from concourse.bass_utils import run_bass_kernel_spmd
    spec = Spec(inputs)
    nc = build(spec, debug=False)
    in_maps = make_in_maps(spec, np.asarray(inputs["x"], np.float32))
    res = run_bass_kernel_spmd(nc, in_maps, list(range(8)))
    return assemble_out(res.results).astype(np.float32)


# revision 11
# speedup vs baseline: 1.0761x; 1.0761x over previous
"""ConformerBlock Trainium2 kernel (Bass/Tile), 8-core SPMD — v2.

Sharding: core c handles batch b=c//2, sequence half c%2.  Half-1 cores
receive the sequence REVERSED (and reversed conv taps) so that every core's
program is identical: own tokens are positions [0,1024), the query/conv
window is positions [0,1152), conv zero-padding is on the left edge.
Attention keys are order-invariant; the depthwise conv commutes with
reversal when taps are reversed; everything else is per-token.

ffn1 + qkv run redundantly over the full 2048-token batch so attention K/V
need no cross-core communication.

v2 numerics: act_quant's round-trip round(x*s)/s is per-token quantization
noise on a relative grid; it is approximated by a plain bf16 cast (same
relative step 2^-9, scale-invariant), which eliminates all per-token
absmax/scale work.  Ternary weights are exact in bf16; matmuls accumulate
in fp32 PSUM.  Validated end-to-end in numpy: rel err 2.5e-3 vs reference.

v2 layout: BitLinear branches run FEATURE-major internally (out = W @ x
via lhsT=W-chunk stationary, rhs=x^T streaming), so the FFN hidden is
never transposed; snake runs with per-partition (=per-feature) activation
scales.  Only branch inputs/outputs are transposed (bf16, 1 cyc/row).
The depthwise conv consumes the feature-major GLU output directly.
"""

from contextlib import ExitStack

import ml_dtypes
import numpy as np

import concourse.bass as bass
import concourse.mybir as mybir
import concourse.tile as tile
from concourse.bass import ts
from concourse.masks import make_identity
import json as _json


def _fix_bir(nc):
    """This container's walrus allows at most ONE sem wait per instruction.
    Hoist surplus waits: for engine instructions onto injected same-engine
    NoOps; for DMACopies onto prepended 1-element dummy copies on the same
    queue (ring order gates the real transfer, identical semantics)."""
    orig = nc.to_json_bytes

    def patched():
        import copy as _copy
        data = _json.loads(orig())
        used = set()
        for fn in data["functions"]:
            for bb in fn["blocks"]:
                for ins in bb["instructions"]:
                    si2 = ins.get("sync_info") or {}
                    for w in (si2.get("on_wait") or []):
                        used.add(w.get("id", 0))
                    for u in (si2.get("on_update") or []):
                        used.add(u.get("id", 0))
        scratch_sem = max(used) + 1 if used else 60
        k = 0
        for fn in data["functions"]:
            for bb in fn["blocks"]:
                out = []
                for ins in bb["instructions"]:
                    si = ins.get("sync_info")
                    ow = (si or {}).get("on_wait") or []
                    if len(ow) > 1:
                        if ins.get("opcode") == "DMACopy":
                            for w in ow[:-1]:
                                k += 1
                                d = _copy.deepcopy(ins)
                                d["name"] = f"W-{k}"
                                d["sync_info"] = {
                                    "on_wait": [w],
                                    "on_update": [{
                                        "ant_name": "WFIX_scratch",
                                        "id": scratch_sem,
                                        "sync_type": "semaphore",
                                        "update_mode": "sem-inc",
                                        "update_value": 1}]}
                                for ap in list(d.get("ins", [])) + list(d.get("outs", [])):
                                    if isinstance(ap, dict) and "ap" in ap:
                                        ap["ap"] = [[s, 1] for s, _ in ap["ap"]]
                                out.append(d)
                            si["on_wait"] = [ow[-1]]
                        else:
                            for w in ow[:-1]:
                                k += 1
                                nop = {"name": f"W-{k}",
                                       "engine": ins["engine"],
                                       "opcode": "NoOp", "ins": [],
                                       "outs": [],
                                       "sync_info": {"on_wait": [w]}}
                                if "debug" in ins:
                                    nop["debug"] = ins["debug"]
                                out.append(nop)
                            si["on_wait"] = [ow[-1]]
                    out.append(ins)
                bb["instructions"] = out
        return _json.dumps(data).encode()

    nc.to_json_bytes = patched
    return nc


ml_bf16 = ml_dtypes.bfloat16

P = 128
T = 2048          # tokens per batch
D = 512           # model dim
FF = 2048         # ffn hidden
H = 8             # heads
HD = 64           # head dim
KW = 31           # conv kernel
QN = 1152         # per-core query window (9 tiles), covers own 1024 + halo 15
OWN = 1024
NT = T // P       # 16
NQ = QN // P      # 9
NO = OWN // P     # 8
ND = D // P       # 4
NF = FF // P      # 16
CH = 512          # token chunk (matmul N)
CIN_W = 15 + 1039 + 2  # conv input row: cols j <-> token j-15; pad to 1056

F32 = mybir.dt.float32
BF16 = mybir.dt.bfloat16
AX = mybir.AxisListType.X
OP = mybir.AluOpType
AF = mybir.ActivationFunctionType


# ---------------------------------------------------------------- host prep

def np_w_quant(w):
    """Host replica of reference w_quant: (ternary int8, descale 1/s)."""
    s = np.float32(1.0) / np.clip(
        np.abs(w).mean(dtype=np.float32), np.float32(1e-5), None
    ).astype(np.float32)
    q = np.clip(np.round(w * s), -1, 1).astype(np.int8)
    return q, np.float32(1.0) / s


class Spec:
    """Host-side preprocessing of all parameters (shared across cores)."""

    def __init__(self, inp):
        f32 = np.float32

        def prep(w, kt):
            # ternary, transposed to lhsT layout [K_in, M_out] -> [kt,128,M]
            q, dsc = np_w_quant(np.asarray(w, f32))
            wt = np.ascontiguousarray(q.T).astype(ml_bf16)
            return wt.reshape(kt, P, wt.shape[1]), f32(dsc)

        self.w1a, self.d_w1a = prep(inp["ff1_w1"], ND)    # [4,128,2048]
        self.w2a, self.d_w2a = prep(inp["ff1_w2"], NF)    # [16,128,512]
        self.w1b, self.d_w1b = prep(inp["ff2_w1"], ND)
        self.w2b, self.d_w2b = prep(inp["ff2_w2"], NF)
        self.pw1, self.d_pw1 = prep(inp["pw1_w"], ND)     # [4,128,1024]
        self.pw2, self.d_pw2 = prep(inp["pw2_w"], ND)     # [4,128,512]

        ipw = np.asarray(inp["in_proj_w"], f32)           # [1536, 512]
        self.wqk = np.ascontiguousarray(ipw[: 2 * D].T).astype(ml_bf16).reshape(ND, P, 2 * D)
        self.wv = np.ascontiguousarray(ipw[2 * D:].T).astype(ml_bf16).reshape(ND, P, D)
        self.opw = np.ascontiguousarray(np.asarray(inp["out_proj_w"], f32).T).astype(ml_bf16).reshape(ND, P, D)

        self.ipb = np.asarray(inp["in_proj_b"], f32)
        self.opb = np.asarray(inp["out_proj_b"], f32)
        self.has_ipb = bool(np.any(self.ipb != 0))
        self.has_opb = bool(np.any(self.opb != 0))

        self.nw = {}
        self.has_nw = {}
        for k in ("ff1_norm_w", "attn_norm_w", "conv_norm_w", "ff2_norm_w",
                  "final_norm_w"):
            w = np.asarray(inp[k], f32)
            self.nw[k] = w
            self.has_nw[k] = bool(np.any(w != 1.0))

        def snake(la, lb, dsc):
            # a_eff = a*dsc (sin scale, folded descale); sqivb = sqrt(ivb)
            a = np.exp(np.asarray(la, f32)).astype(f32)
            ivb = (f32(1.0) / (np.exp(np.asarray(lb, f32)) + f32(1e-9))).astype(f32)
            has_a = bool(np.any(a != 1.0))
            has_ivb = bool(np.any(np.abs(ivb - 1.0) > 1e-7))
            return (a * dsc).astype(f32), np.sqrt(ivb).astype(f32), has_a, has_ivb

        self.a1, self.sqivb1, self.has_a1, self.has_ivb1 = snake(
            inp["ff1_a"], inp["ff1_b"], self.d_w1a)
        self.a2, self.sqivb2, self.has_a2, self.has_ivb2 = snake(
            inp["ff2_a"], inp["ff2_b"], self.d_w1b)
        self.a3, self.sqivb3, self.has_a3, self.has_ivb3 = snake(
            inp["snake_a"], inp["snake_b"], np.float32(1.0))

        # depthwise conv folded with batchnorm:
        # y = conv(glu)*A + B,  A = g*rsqrt(v+1e-5), B = (dwb-m)*A + b
        A = (np.asarray(inp["bn_g"], f32)
             / np.sqrt(np.asarray(inp["bn_v"], f32) + f32(1e-5))).astype(f32)
        Bb = ((np.asarray(inp["dw_b"], f32) - np.asarray(inp["bn_m"], f32)) * A
              + np.asarray(inp["bn_b"], f32)).astype(f32)
        dw = np.asarray(inp["dw_w"], f32)[:, 0, :]        # [512, 31]
        self.wA = (dw * A[:, None]).astype(f32)           # [512, 31]
        self.convB = Bb.reshape(ND, P)                    # [4, 128]


# ------------------------------------------------------------- device build

class Ctx:
    def __init__(self, nc, tc, st):
        self.nc, self.tc, self.st = nc, tc, st


def _batched_stats(c, pool, src, ntiles):
    """Per-tile sum(x^2) over token tiles of src [128, ntiles, 512]."""
    nc = c.nc
    ssum = pool.tile([P, ntiles], F32, tag="st_ssum", name="st_ssum")
    sq = pool.tile([P, D], F32, tag="st_sq", name="st_sq", bufs=2)
    for i in range(ntiles):
        nc.vector.tensor_mul(sq, src[:, i, :], src[:, i, :])
        nc.vector.tensor_reduce(ssum[:, i:i + 1], sq, AX, OP.add)
    return ssum


def _rsqrt_newton(c, pool, ssum, n_el, eps=1e-6):
    """rs = rsqrt(ssum/n_el + eps) with one Newton step."""
    nc = c.nc
    n = ssum.shape[-1]
    m = pool.tile([P, n], F32, tag="st_m", name="st_m")
    nc.vector.tensor_scalar(out=m, in0=ssum, scalar1=1.0 / n_el, scalar2=eps,
                            op0=OP.mult, op1=OP.add)
    rc = pool.tile([P, n], F32, tag="st_rc", name="st_rc")
    nc.vector.reciprocal(rc, m)
    rs = pool.tile([P, n], F32, tag="st_rs", name="st_rs")
    nc.scalar.activation(out=rs, in_=rc, func=AF.Sqrt)
    t1 = pool.tile([P, n], F32, tag="st_t1", name="st_t1")
    nc.vector.tensor_mul(t1, rs, rs)
    nc.vector.tensor_mul(t1, t1, m)
    nc.vector.tensor_scalar(out=t1, in0=t1, scalar1=-0.5, scalar2=1.5,
                            op0=OP.mult, op1=OP.add)
    nc.vector.tensor_mul(rs, rs, t1)
    return rs


def _norm_cast_T(c, pools, src, ntiles, nw_b, ident, out_tag="xqT"):
    """rmsnorm (approx act_quant = bf16 cast) + transpose.
    src [128, ntiles, 512] f32 -> xqT [128, 4, ntiles*128] bf16."""
    nc = c.nc
    pool, scratch, pst = pools
    ssum = _batched_stats(c, pool, src, ntiles)
    rs = _rsqrt_newton(c, pool, ssum, D)
    xqT = pool.tile([P, ND, ntiles * P], BF16, tag=out_tag, name=out_tag)
    for i in range(ntiles):
        xq = scratch.tile([P, D], BF16, tag="nc_xq", name="nc_xq")
        if nw_b is None:
            nc.vector.tensor_scalar_mul(xq, src[:, i, :], rs[:, i:i + 1])
        else:
            nc.vector.scalar_tensor_tensor(
                out=xq, in0=src[:, i, :], scalar=rs[:, i:i + 1], in1=nw_b,
                op0=OP.mult, op1=OP.mult)
        pt = pst.tile([P, ND, P], BF16, tag="pt", name="pt")
        for dt in range(ND):
            nc.tensor.transpose(pt[:, dt, :], xq[:, ts(dt, P)], ident)
        nc.vector.tensor_copy(xqT[:, :, ts(i, P)], pt)
    return xqT


def _ffn(c, pools, src, ntiles, w1, w2, d1, d2, a_fm, sqivb_fm, has_a,
         has_ivb, nw_b, ident, resid_scale, dst):
    """dst = src + resid_scale * ffn(src).  Feature-major hidden.
    src/dst: [128, ntiles, 512] f32 token-major."""
    nc = c.nc
    pool, scratch, pst, psH, psO = pools
    nch = ntiles * P // CH
    xqT = _norm_cast_T(c, (pool, scratch, pst), src, ntiles, nw_b, ident)
    d1f = float(d1)
    dof = float(d2) * float(resid_scale)
    for j in range(nch):
        hq = scratch.tile([P, NF, CH], BF16, tag="ffn_hq", name="ffn_hq")
        for g in range(NF // 2):
            ph = psH.tile([P, 2, CH], F32, tag="ph", name="ph")
            for f2 in range(2):
                fc = 2 * g + f2
                for kt in range(ND):
                    nc.tensor.matmul(ph[:, f2, :], w1[:, kt, ts(fc, P)],
                                     xqT[:, kt, ts(j, CH)],
                                     start=(kt == 0), stop=(kt == ND - 1))
            phv = ph.rearrange("p a b -> p (a b)")
            sn = scratch.tile([P, 2 * CH], F32, tag="ffn_sn", name="ffn_sn")
            if not has_a:
                nc.scalar.activation(out=sn, in_=phv, func=AF.Sin, scale=d1f)
            else:
                for f2 in range(2):
                    fc = 2 * g + f2
                    nc.scalar.activation(out=sn[:, ts(f2, CH)],
                                         in_=ph[:, f2, :], func=AF.Sin,
                                         scale=a_fm[:, fc:fc + 1])
            sq = scratch.tile([P, 2 * CH], F32, tag="ffn_sq", name="ffn_sq")
            if not has_ivb:
                nc.scalar.activation(out=sq, in_=sn, func=AF.Square)
            else:
                for f2 in range(2):
                    fc = 2 * g + f2
                    nc.scalar.activation(out=sq[:, ts(f2, CH)],
                                         in_=sn[:, ts(f2, CH)],
                                         func=AF.Square,
                                         scale=sqivb_fm[:, fc:fc + 1])
            hqv = hq[:, 2 * g:2 * g + 2, :].rearrange("p a b -> p (a b)")
            nc.vector.scalar_tensor_tensor(out=hqv, in0=phv, scalar=d1f,
                                           in1=sq, op0=OP.mult, op1=OP.add)
        for dc in range(ND):
            p2 = psO.tile([P, CH], F32, tag="p2", name="p2")
            for kt in range(NF):
                nc.tensor.matmul(p2, w2[:, kt, ts(dc, P)], hq[:, kt, :],
                                 start=(kt == 0), stop=(kt == NF - 1))
            ob = scratch.tile([P, CH], BF16, tag="ffn_ob", name="ffn_ob")
            nc.vector.tensor_copy(ob, p2)
            pt = pst.tile([P, ND, P], BF16, tag="pt", name="pt")
            for tc in range(4):
                nc.tensor.transpose(pt[:, tc, :], ob[:, ts(tc, P)], ident)
            for tc in range(4):
                i = 4 * j + tc
                nc.vector.scalar_tensor_tensor(
                    out=dst[:, i, ts(dc, P)], in0=pt[:, tc, :], scalar=dof,
                    in1=src[:, i, ts(dc, P)], op0=OP.mult, op1=OP.add)


def build(spec: Spec, debug=False):
    nc = bass.Bass()
    st = spec
    if st.has_ipb:
        raise NotImplementedError("nonzero in_proj_b not supported")

    # ---- dram params
    x_d = nc.declare_dram_parameter("x", [T, D], F32, isOutput=False)
    w_names = {}
    for nm, arr in [("w1a", st.w1a), ("w2a", st.w2a), ("w1b", st.w1b),
                    ("w2b", st.w2b), ("pw1", st.pw1), ("pw2", st.pw2),
                    ("wqk", st.wqk), ("wv", st.wv), ("opw", st.opw)]:
        w_names[nm] = nc.declare_dram_parameter(nm, list(arr.shape), BF16,
                                                isOutput=False)
    wA_d = nc.declare_dram_parameter("wA", [ND, P, KW], F32, isOutput=False)
    convB_d = nc.declare_dram_parameter("convB", [ND, P], F32, isOutput=False)
    vec_d = {}
    for nm, need, n in [("a1", st.has_a1, FF), ("sqivb1", st.has_ivb1, FF),
                        ("a2", st.has_a2, FF), ("sqivb2", st.has_ivb2, FF),
                        ("a3", st.has_a3, D), ("sqivb3", st.has_ivb3, D),
                        ("opb", st.has_opb, D)]:
        if need:
            vec_d[nm] = nc.declare_dram_parameter(nm, [n], F32, isOutput=False)
    nwflags = ["ff1_norm_w", "attn_norm_w", "conv_norm_w", "ff2_norm_w",
               "final_norm_w"]
    for k in nwflags:
        if st.has_nw[k]:
            vec_d[k] = nc.declare_dram_parameter(k, [D], F32, isOutput=False)

    out_d = nc.declare_dram_parameter("out", [OWN, D], F32, isOutput=True)
    if debug:
        dbg1 = nc.declare_dram_parameter("dbg_x1", [T, D], F32, isOutput=True)
        dbg2 = nc.declare_dram_parameter("dbg_x2", [QN, D], F32, isOutput=True)
        dbg3 = nc.declare_dram_parameter("dbg_x3", [OWN, D], F32, isOutput=True)

    def bcast_load(pool, dram_ap, n, tag):
        t = pool.tile([P, n], F32, tag=tag, name=tag)
        src = bass.AP(tensor=dram_ap.tensor, offset=dram_ap.offset,
                      ap=[[0, P]] + dram_ap.ap)
        nc.sync.dma_start(out=t, in_=src)
        return t

    def fm_load(pool, dram_ap, ncol, tag):
        # [ncol*128] vector -> [128, ncol] feature-major tile
        t = pool.tile([P, ncol], F32, tag=tag, name=tag)
        nc.sync.dma_start(out=t, in_=dram_ap.rearrange("(a p) -> p a", p=P))
        return t

    def load_w(pool, nm):
        arr = getattr(st, nm)
        t = pool.tile([P, arr.shape[0], arr.shape[2]], BF16,
                      tag=f"w_{nm}", name=f"w_{nm}")
        for kt in range(arr.shape[0]):
            nc.sync.dma_start(out=t[:, kt, :], in_=w_names[nm][kt])
        return t

    with tile.TileContext(nc) as tc:
        c = Ctx(nc, tc, st)
        with ExitStack() as es:
            glob = es.enter_context(tc.tile_pool(name="glob", bufs=1))
            pst = es.enter_context(tc.tile_pool(name="pst", bufs=2,
                                                space="PSUM"))

            ident = glob.tile([P, P], BF16)
            make_identity(nc, ident)

            a1_fm = fm_load(glob, vec_d["a1"][:], NF, "a1fm") if st.has_a1 else None
            sqivb1_fm = fm_load(glob, vec_d["sqivb1"][:], NF, "sqivb1fm") if st.has_ivb1 else None
            a2_fm = fm_load(glob, vec_d["a2"][:], NF, "a2fm") if st.has_a2 else None
            sqivb2_fm = fm_load(glob, vec_d["sqivb2"][:], NF, "sqivb2fm") if st.has_ivb2 else None
            a3_fm = fm_load(glob, vec_d["a3"][:], ND, "a3fm") if st.has_a3 else None
            sqivb3_fm = fm_load(glob, vec_d["sqivb3"][:], ND, "sqivb3fm") if st.has_ivb3 else None
            opb_b = bcast_load(glob, vec_d["opb"][:], D, "opbb") if st.has_opb else None
            nw_b = {k: (bcast_load(glob, vec_d[k][:], D, f"nw_{k}")
                        if st.has_nw[k] else None) for k in nwflags}

            # persistent residual-stream tiles (updated in place)
            X = glob.tile([P, NT, D], F32)     # x, then x1 after phase A
            x2 = glob.tile([P, NQ, D], F32)    # x2, then x3/x4 in [:, :8, :]

            xr = x_d[:].rearrange("(t p) d -> p t d", p=P)
            for i in range(NT):
                nc.sync.dma_start(out=X[:, i, :], in_=xr[:, i, :])

            # ---------------- phase A: ffn1 over full batch; X <- x1
            with tc.tile_pool(name="poolA", bufs=1) as pool, \
                 tc.tile_pool(name="scrA", bufs=2) as scratch, \
                 tc.tile_pool(name="psHA", bufs=2, space="PSUM") as psH, \
                 tc.tile_pool(name="psOA", bufs=2, space="PSUM") as psO:
                w1 = load_w(pool, "w1a")
                w2 = load_w(pool, "w2a")
                _ffn(c, (pool, scratch, pst, psH, psO), X, NT,
                     w1, w2, st.d_w1a, st.d_w2a, a1_fm, sqivb1_fm,
                     st.has_a1, st.has_ivb1, nw_b["ff1_norm_w"], ident,
                     0.5, X)
            if debug:
                d1r = dbg1[:].rearrange("(t p) d -> p t d", p=P)
                for i in range(NT):
                    nc.sync.dma_start(out=d1r[:, i, :], in_=X[:, i, :])

            # ---------------- phase B: attention -> x2 (window [0,1152))
            with tc.tile_pool(name="poolB", bufs=1) as pool, \
                 tc.tile_pool(name="scrB", bufs=2) as scratch:
                wqk = load_w(pool, "wqk")
                wv = load_w(pool, "wv")
                opw = load_w(pool, "opw")

                # B1: rmsnorm(x1) -> bf16, transposed
                xn2T = _norm_cast_T(c, (pool, scratch, pst), X, NT,
                                    nw_b["attn_norm_w"], ident,
                                    out_tag="xn2T")

                # B2: q,k feature-major; B3: v token-major + ones cols
                qkT = pool.tile([P, H, T], BF16)
                V = pool.tile([P, NT, H * (HD + 1)], BF16)
                nc.vector.memset(V, 1.0)
                with tc.tile_pool(name="psB", bufs=1, space="PSUM") as psB, \
                     tc.tile_pool(name="psB2", bufs=2, space="PSUM") as psB2:
                    for mt in range(H):
                        pq = psB.tile([P, ND, D], F32, tag="ps_qk",
                                      name="ps_qk")
                        for nc_ in range(ND):
                            for kt in range(ND):
                                nc.tensor.matmul(
                                    pq[:, nc_, :], wqk[:, kt, ts(mt, P)],
                                    xn2T[:, kt, ts(nc_, D)],
                                    start=(kt == 0), stop=(kt == ND - 1))
                        pqv = pq.rearrange("p a b -> p (a b)")
                        nc.vector.tensor_copy(qkT[:, mt, :], pqv)
                    for i in range(NT):
                        pv = psB2.tile([P, D], F32, tag="ps_v", name="ps_v")
                        for kt in range(ND):
                            nc.tensor.matmul(pv, xn2T[:, kt, ts(i, P)],
                                             wv[:, kt, :],
                                             start=(kt == 0),
                                             stop=(kt == ND - 1))
                        vv = V[:, i, :].rearrange("p (h z) -> p h z", z=HD + 1)
                        nc.vector.tensor_copy(
                            vv[:, :, 0:HD],
                            pv.rearrange("p (h z) -> p h z", z=HD))

                # B4: attention per head over query window [0, QN).
                # scores -> exp -> PV interleaved at 2-key-tile granularity
                # so the PE never idles waiting for exp (keeps HAM warm) and
                # exp hides under the matmul stream.
                On = pool.tile([P, ND, QN], BF16)
                QCH = [(0, 512), (512, 512), (1024, 128)]
                with tc.tile_pool(name="psS", bufs=2, space="PSUM") as psS, \
                     tc.tile_pool(name="psO", bufs=2, space="PSUM") as psO, \
                     tc.tile_pool(name="drB", bufs=2, space="DRAM") as drB:
                    for h in range(H):
                        kf_t, kf_o = ND + h // 2, (h % 2) * HD
                        qf_t, qf_o = h // 2, (h % 2) * HD
                        for (q0, qw) in QCH:
                            po = psO.tile([HD + 1, 512], F32, tag="ps_o",
                                          name="ps_o")
                            for g in range(8):
                                ps = psS.tile([P, 2, 512], F32, tag="ps_s",
                                              name="ps_s")
                                for k2 in range(2):
                                    kt = 2 * g + k2
                                    nc.tensor.matmul(
                                        ps[:, k2, :qw],
                                        qkT[kf_o:kf_o + HD, kf_t, ts(kt, P)],
                                        qkT[qf_o:qf_o + HD, qf_t, q0:q0 + qw],
                                        start=True, stop=True)
                                PTg = scratch.tile([P, 2, 512], BF16,
                                                   tag="PT", name="PT",
                                                   bufs=3)
                                nc.scalar.activation(
                                    out=PTg[:, :, :qw], in_=ps[:, :, :qw],
                                    func=AF.Exp, scale=0.125)
                                for k2 in range(2):
                                    kt = 2 * g + k2
                                    nc.tensor.matmul(
                                        po[:, :qw],
                                        V[:, kt,
                                          h * (HD + 1):(h + 1) * (HD + 1)],
                                        PTg[:, k2, :qw],
                                        start=(kt == 0), stop=(kt == NT - 1),
                                        skip_group_check=True)
                            # normalize rows 0:64 by row 64 (denominator):
                            # reciprocal on scalar engine, partition-broadcast
                            # via SBUF->SBUF DMA, multiply on DVE.
                            dn = scratch.tile([1, 512], F32, tag="dn",
                                              name="dn")
                            nc.vector.reciprocal(dn[:, :qw],
                                                 po[HD:HD + 1, :qw])
                            bounce = drB.tile([1, 512], F32, tag="bnc",
                                              name="bnc")
                            nc.sync.dma_start(out=bounce[:, :qw],
                                              in_=dn[:, :qw])
                            rd = scratch.tile([HD, 512], F32, tag="rd",
                                              name="rd")
                            bap = bass.AP(
                                tensor=bounce.tensor, offset=bounce.offset,
                                ap=[[0, HD]] + bounce[:, :qw].ap[1:])
                            nc.sync.dma_start(out=rd[:, :qw], in_=bap)
                            nc.vector.tensor_mul(
                                On[qf_o:qf_o + HD, qf_t, q0:q0 + qw],
                                po[0:HD, :qw], rd[:, :qw])

                # B6: out-proj + residual -> x2
                with tc.tile_pool(name="psB6", bufs=2, space="PSUM") as psB6:
                    for i in range(NQ):
                        pp = psB6.tile([P, D], F32, tag="ps_op", name="ps_op")
                        for kt in range(ND):
                            nc.tensor.matmul(pp, On[:, kt, ts(i, P)],
                                             opw[:, kt, :],
                                             start=(kt == 0),
                                             stop=(kt == ND - 1))
                        if opb_b is not None:
                            tmp = scratch.tile([P, D], F32, tag="b6_t",
                                               name="b6_t")
                            nc.vector.tensor_add(tmp, pp, opb_b)
                            nc.vector.tensor_add(x2[:, i, :], tmp, X[:, i, :])
                        else:
                            nc.vector.scalar_tensor_tensor(
                                out=x2[:, i, :], in0=pp, scalar=1.0,
                                in1=X[:, i, :], op0=OP.mult, op1=OP.add)
            if debug:
                d2r = dbg2[:].rearrange("(t p) d -> p t d", p=P)
                for i in range(NQ):
                    nc.sync.dma_start(out=d2r[:, i, :], in_=x2[:, i, :])

            # ---------------- phase C: conv branch; x2[:, :8] <- x3
            with tc.tile_pool(name="poolC", bufs=1) as pool, \
                 tc.tile_pool(name="scrC", bufs=2) as scratch:
                pw1 = load_w(pool, "pw1")
                pw2 = load_w(pool, "pw2")
                wA_sb = pool.tile([P, ND, KW], F32)
                for kt in range(ND):
                    nc.sync.dma_start(out=wA_sb[:, kt, :], in_=wA_d[kt])
                convB_sb = pool.tile([P, ND], F32)
                nc.sync.dma_start(out=convB_sb,
                                  in_=convB_d[:].rearrange("a p -> p a"))

                xq3T = _norm_cast_T(c, (pool, scratch, pst), x2, NQ,
                                    nw_b["conv_norm_w"], ident,
                                    out_tag="xq3T")
                cin = pool.tile([P, ND, CIN_W], F32)
                nc.gpsimd.memset(cin, 0.0)

                d1f = float(st.d_pw1)
                QCH = [(0, 512), (512, 512), (1024, 128)]
                with tc.tile_pool(name="psC1", bufs=2, space="PSUM") as psC1, \
                     tc.tile_pool(name="psC2", bufs=2, space="PSUM") as psC2:
                    # pw1 + GLU, feature-major straight into cin
                    for (q0, qw) in QCH:
                        w = min(qw, 1054 - (15 + q0))
                        for ct in range(ND):
                            pp = psC1.tile([P, 2, CH], F32, tag="ps_pw1",
                                           name="ps_pw1")
                            for s, fc in ((0, ct), (1, 4 + ct)):
                                for kt in range(ND):
                                    nc.tensor.matmul(
                                        pp[:, s, :qw], pw1[:, kt, ts(fc, P)],
                                        xq3T[:, kt, q0:q0 + qw],
                                        start=(kt == 0), stop=(kt == ND - 1))
                            sig = scratch.tile([P, CH], F32, tag="c_sig",
                                               name="c_sig")
                            nc.scalar.activation(out=sig[:, :qw],
                                                 in_=pp[:, 1, :qw],
                                                 func=AF.Sigmoid, scale=d1f)
                            nc.vector.scalar_tensor_tensor(
                                out=cin[:, ct, 15 + q0:15 + q0 + w],
                                in0=pp[:, 0, :w], scalar=d1f,
                                in1=sig[:, :w], op0=OP.mult, op1=OP.mult)
                    # depthwise conv + folded bn (DVE)
                    acc = pool.tile([P, ND, OWN], F32)
                    for ct in range(ND):
                        nc.vector.tensor_scalar(
                            out=acc[:, ct, :], in0=cin[:, ct, 0:OWN],
                            scalar1=wA_sb[:, ct, 0:1],
                            scalar2=convB_sb[:, ct:ct + 1],
                            op0=OP.mult, op1=OP.add)
                        for k in range(1, KW):
                            nc.vector.scalar_tensor_tensor(
                                out=acc[:, ct, :], in0=cin[:, ct, k:k + OWN],
                                scalar=wA_sb[:, ct, k:k + 1],
                                in1=acc[:, ct, :], op0=OP.mult, op1=OP.add)
                    # snake (feature-major) -> quantized (bf16) pw2 input
                    zq = pool.tile([P, ND, OWN], BF16)
                    for ct in range(ND):
                        sn = scratch.tile([P, OWN], F32, tag="c_sn",
                                          name="c_sn")
                        sq = scratch.tile([P, OWN], F32, tag="c_sq",
                                          name="c_sq")
                        if st.has_a3:
                            nc.scalar.activation(out=sn, in_=acc[:, ct, :],
                                                 func=AF.Sin,
                                                 scale=a3_fm[:, ct:ct + 1])
                        else:
                            nc.scalar.activation(out=sn, in_=acc[:, ct, :],
                                                 func=AF.Sin)
                        if st.has_ivb3:
                            nc.scalar.activation(out=sq, in_=sn,
                                                 func=AF.Square,
                                                 scale=sqivb3_fm[:, ct:ct + 1])
                        else:
                            nc.scalar.activation(out=sq, in_=sn,
                                                 func=AF.Square)
                        nc.vector.tensor_add(zq[:, ct, :], acc[:, ct, :], sq)
                    # pw2 (feature-major) + transpose + residual
                    dof = float(st.d_pw2)
                    for j2 in range(2):
                        for dc in range(ND):
                            p2 = psC2.tile([P, CH], F32, tag="ps_pw2",
                                           name="ps_pw2")
                            for kt in range(ND):
                                nc.tensor.matmul(
                                    p2, pw2[:, kt, ts(dc, P)],
                                    zq[:, kt, ts(j2, CH)],
                                    start=(kt == 0), stop=(kt == ND - 1))
                            ob = scratch.tile([P, CH], BF16, tag="c_ob",
                                              name="c_ob")
                            nc.vector.tensor_copy(ob, p2)
                            pt = pst.tile([P, ND, P], BF16, tag="pt",
                                          name="pt")
                            for tc_ in range(4):
                                nc.tensor.transpose(pt[:, tc_, :],
                                                    ob[:, ts(tc_, P)], ident)
                            for tc_ in range(4):
                                i8 = 4 * j2 + tc_
                                nc.vector.scalar_tensor_tensor(
                                    out=x2[:, i8, ts(dc, P)],
                                    in0=pt[:, tc_, :], scalar=dof,
                                    in1=x2[:, i8, ts(dc, P)],
                                    op0=OP.mult, op1=OP.add)
            if debug:
                d3r = dbg3[:].rearrange("(t p) d -> p t d", p=P)
                for i in range(NO):
                    nc.sync.dma_start(out=d3r[:, i, :], in_=x2[:, i, :])

            # ---------------- phase D: ffn2; x2[:, :8] <- x4
            with tc.tile_pool(name="poolD", bufs=1) as pool, \
                 tc.tile_pool(name="scrD", bufs=2) as scratch, \
                 tc.tile_pool(name="psHD", bufs=2, space="PSUM") as psH, \
                 tc.tile_pool(name="psOD", bufs=2, space="PSUM") as psO:
                w1 = load_w(pool, "w1b")
                w2 = load_w(pool, "w2b")
                _ffn(c, (pool, scratch, pst, psH, psO), x2, NO,
                     w1, w2, st.d_w1b, st.d_w2b, a2_fm, sqivb2_fm,
                     st.has_a2, st.has_ivb2, nw_b["ff2_norm_w"], ident,
                     0.5, x2)

            # ---------------- phase E: final rmsnorm -> out
            with tc.tile_pool(name="poolE", bufs=1) as pool, \
                 tc.tile_pool(name="scrE", bufs=2) as scratch:
                ssum5 = _batched_stats(c, pool, x2, NO)
                rs5 = _rsqrt_newton(c, pool, ssum5, D)
                outr = out_d[:].rearrange("(t p) d -> p t d", p=P)
                for i in range(NO):
                    o = scratch.tile([P, D], F32, tag="e_o", name="e_o")
                    if nw_b["final_norm_w"] is None:
                        nc.vector.tensor_scalar_mul(o, x2[:, i, :],
                                                    rs5[:, i:i + 1])
                    else:
                        nc.vector.scalar_tensor_tensor(
                            out=o, in0=x2[:, i, :], scalar=rs5[:, i:i + 1],
                            in1=nw_b["final_norm_w"], op0=OP.mult,
                            op1=OP.mult)
                    nc.sync.dma_start(out=outr[:, i, :], in_=o)

    return _fix_bir(nc)


# ------------------------------------------------------------------ runner

def make_in_maps(spec: Spec, x_full):
    """x_full: [4, 2048, 512] f32.  Returns per-core input maps."""
    maps = []
    shared = {"w1a": spec.w1a, "w2a": spec.w2a, "w1b": spec.w1b,
              "w2b": spec.w2b, "pw1": spec.pw1, "pw2": spec.pw2,
              "wqk": spec.wqk, "wv": spec.wv, "opw": spec.opw}
    wA_f = spec.wA.reshape(ND, P, KW)
    wA_r = np.ascontiguousarray(spec.wA[:, ::-1]).reshape(ND, P, KW)
    opt = {}
    for nm, need, arr in [("a1", spec.has_a1, spec.a1),
                          ("sqivb1", spec.has_ivb1, spec.sqivb1),
                          ("a2", spec.has_a2, spec.a2),
                          ("sqivb2", spec.has_ivb2, spec.sqivb2),
                          ("a3", spec.has_a3, spec.a3),
                          ("sqivb3", spec.has_ivb3, spec.sqivb3),
                          ("opb", spec.has_opb, spec.opb)]:
        if need:
            opt[nm] = arr
    for k, need in spec.has_nw.items():
        if need:
            opt[k] = spec.nw[k]
    for cid in range(8):
        b, flip = cid // 2, cid % 2
        xb = x_full[b] if not flip else np.ascontiguousarray(x_full[b][::-1])
        m = {"x": np.asarray(xb, np.float32),
             "wA": wA_r if flip else wA_f,
             "convB": spec.convB, **shared, **opt}
        maps.append(m)
    return maps


def assemble_out(results):
    """results: list of 8 dicts with 'out' [1024, 512]."""
    y = np.zeros((4, T, D), np.float32)
    for cid in range(8):
        b, flip = cid // 2, cid % 2
        o = results[cid]["out"]
        if flip:
            y[b, OWN:] = o[::-1]
        else:
            y[b, :OWN] = o
    return y


# ------------------------------------------------------------------ entry

def kernel(**inputs):
    """Full-input entry point: shards across 8 NeuronCores internally."""
    from concourse.bass_utils import run_bass_kernel_spmd
    spec = Spec(inputs)
    nc = build(spec, debug=False)
    in_maps = make_in_maps(spec, np.asarray(inputs["x"], np.float32))
    res = run_bass_kernel_spmd(nc, in_maps, list(range(8)))
    return assemble_out(res.results).astype(np.float32)
